# revision 14
# baseline (speedup 1.0000x reference)
"""Trainium2 Bass kernel for nn_OPTAttention (OPT attention with SmoothK fake-quant).

Sharding: tensor-parallel over heads for QKV+attention (4 heads/core x 8 cores),
AllToAll to token-shard the attention output, token-parallel out_proj
(512 tokens/core). Matmuls run as float32r (full-rate fp32 mode on the PE).

Self-contained: hardcodes B=4, T=1024, E=2048, H=32.
"""
import numpy as np

import concourse.bass as bass
import concourse.bacc as bacc
import concourse.mybir as mybir
import concourse.tile as tile

B, T, E, H = 4, 1024, 2048, 32
D = E // H            # 64
P = 128
NCORES = 8
HL = H // NCORES      # 4 heads per core
HD = HL * D           # 256 head-dim columns per core
TOK = B * T           # 4096 tokens
KSUB = E // P         # 16 contraction subtiles
NCHUNK = TOK // P     # 32 token chunks
CPB = T // P          # 8 chunks per batch
TSLICE = TOK // NCORES  # 512 tokens per core for out_proj
MAXQ = 255.0
MAGIC = float(np.float32(1.5 * 2 ** 23))
INV_MAXQ = float(np.float32(1.0 / 255.0))

F32 = mybir.dt.float32
F32R = mybir.dt.float32r
AX = mybir.AxisListType
OP = mybir.AluOpType
ACTF = mybir.ActivationFunctionType


def _quant_params(nc, sp, mx, mn, shape, tag):
    """Compute fake-quant params from raw max/min tiles. Returns (rs, scale, negz, mhz)."""
    scale = sp.tile(shape, F32, tag=f"{tag}_scale")
    cond = sp.tile(shape, F32, tag=f"{tag}_cond")
    rs = sp.tile(shape, F32, tag=f"{tag}_rs")
    negz = sp.tile(shape, F32, tag=f"{tag}_negz")
    mhz = sp.tile(shape, F32, tag=f"{tag}_mhz")
    nc.vector.tensor_scalar(mx, mx, 0.0, None, OP.max)
    nc.vector.tensor_scalar(mn, mn, 0.0, None, OP.min)
    nc.vector.tensor_tensor(scale, mx, mn, OP.subtract)
    nc.vector.tensor_scalar(scale, scale, INV_MAXQ, None, OP.mult)
    nc.vector.tensor_scalar(cond, scale, 0.0, None, OP.is_le)
    nc.vector.tensor_tensor(scale, scale, cond, OP.add)
    nc.vector.reciprocal(rs, scale)
    nc.vector.tensor_tensor(negz, mn, rs, OP.mult)
    nc.vector.tensor_scalar(negz, negz, MAGIC, MAGIC, OP.add, OP.subtract)
    nc.vector.tensor_scalar(mhz, negz, MAXQ, None, OP.add)
    return rs, scale, negz, mhz


def _per_token_quant(nc, sp, x):
    """Fake-quantize x [128, HD] in place, per token row within each head's D cols."""
    xv = x.rearrange("p (h d) -> p h d", d=D)
    mx = sp.tile([P, HL], F32, tag="tq_mx")
    mn = sp.tile([P, HL], F32, tag="tq_mn")
    nc.vector.tensor_reduce(mx, xv, AX.X, OP.max)
    nc.vector.tensor_reduce(mn, xv, AX.X, OP.min)
    rs, scale, negz, mhz = _quant_params(nc, sp, mx, mn, [P, HL], "tq")
    nc.vector.tensor_tensor(xv, xv, rs[:, :, None].to_broadcast(xv.shape), OP.mult)
    nc.vector.tensor_scalar(x, x, MAGIC, MAGIC, OP.add, OP.subtract)
    for h in range(HL):
        hs_ = slice(D * h, D * h + D)
        nc.vector.tensor_scalar(
            x[:, hs_], x[:, hs_], negz[:, h: h + 1], mhz[:, h: h + 1], OP.max, OP.min
        )
    nc.vector.tensor_tensor(xv, xv, scale[:, :, None].to_broadcast(xv.shape), OP.mult)


def build_nc():
    nc = bacc.Bacc("TRN2", target_bir_lowering=False, debug=False, num_devices=NCORES)

    hs_re = nc.dram_tensor("hs_re", [NCHUNK, P, KSUB, P], F32R, kind="ExternalInput")
    wq = nc.dram_tensor("wq", [KSUB, P, HD], F32R, kind="ExternalInput")
    wk = nc.dram_tensor("wk", [KSUB, P, HD], F32R, kind="ExternalInput")
    wv = nc.dram_tensor("wv", [KSUB, P, HD], F32R, kind="ExternalInput")
    bqr = nc.dram_tensor("bqr", [1, HD], F32R, kind="ExternalInput")
    bkr = nc.dram_tensor("bkr", [1, HD], F32R, kind="ExternalInput")
    bvr = nc.dram_tensor("bvr", [1, HD], F32R, kind="ExternalInput")
    wo_re = nc.dram_tensor("wo_re", [KSUB, P, KSUB, P], F32R, kind="ExternalInput")
    bo_re = nc.dram_tensor("bo_re", [P, KSUB], F32, kind="ExternalInput")
    maskband = nc.dram_tensor("maskband", [B, CPB, P, P], F32, kind="ExternalInput")
    out_dram = nc.dram_tensor("out", [KSUB, P, TSLICE], F32, kind="ExternalOutput")

    with tile.TileContext(nc) as tc:
        _build(tc, hs_re.ap(), (wq.ap(), wk.ap(), wv.ap()),
               (bqr.ap(), bkr.ap(), bvr.ap()), wo_re.ap(), bo_re.ap(),
               maskband.ap(), out_dram.ap())
    nc.compile()
    return nc


def _build(tc, hs_re, w_d, b_d, wo_re, bo_re, maskband, out_dram):
    nc = tc.nc
    from contextlib import ExitStack
    from concourse.masks import make_identity

    with ExitStack() as ctx:
        const = ctx.enter_context(tc.tile_pool(name="const", bufs=1))
        dram = ctx.enter_context(tc.tile_pool(name="dram", bufs=1, space="DRAM"))

        ident = const.tile([P, P], F32, tag="ident")
        make_identity(nc, ident)
        ones_f32 = const.tile([P, 1], F32, tag="ones_f32")
        nc.gpsimd.memset(ones_f32, 1.0)
        zeros_f32 = const.tile([P, 384], F32, tag="zeros_f32")
        nc.gpsimd.memset(zeros_f32, 0.0)
        # all-ones f32r block: row slices serve as K=1 matmul lhsT at any base
        ones_all = const.tile([P, P], F32R, tag="ones_all")
        nc.scalar.copy(ones_all, ones_f32[:, 0:1].to_broadcast((P, P)))

        w_sb = []
        b_sb = []
        for i, (wd, bd) in enumerate(zip(w_d, b_d)):
            w = const.tile([P, KSUB, HD], F32R, tag=f"w{i}")
            nc.sync.dma_start(w, wd.rearrange("ko p n -> p ko n"))
            w_sb.append(w)
            bb = const.tile([1, HD], F32R, tag=f"b{i}")
            nc.sync.dma_start(bb, bd)
            b_sb.append(bb)
        bo_sb = const.tile([P, KSUB], F32, tag="bo")
        nc.sync.dma_start(bo_sb, bo_re)

        a2a_in = dram.tile([NCORES, HD, TSLICE], F32R, tag="a2a_in")
        a2a_out = dram.tile([NCORES, HD, TSLICE], F32R, tag="a2a_out")
        scratch = dram.tile([4, HD], F32, tag="scratch")  # stat col->row flatten

        with ExitStack() as actx:
            hsp = actx.enter_context(tc.tile_pool(name="hsp", bufs=2))
            ppool = actx.enter_context(tc.tile_pool(name="ppool", bufs=3, space="PSUM"))
            tpsum = actx.enter_context(tc.tile_pool(name="tpsum", bufs=1, space="PSUM"))
            tmp = actx.enter_context(tc.tile_pool(name="tmp", bufs=3))
            dpool = actx.enter_context(tc.tile_pool(name="dpool", bufs=2))
            spc = actx.enter_context(tc.tile_pool(name="spc", bufs=2))
            spb = actx.enter_context(tc.tile_pool(name="spb", bufs=1))
            bigp = actx.enter_context(tc.tile_pool(name="bigp", bufs=1))
            qkT = actx.enter_context(tc.tile_pool(name="qkT", bufs=2))
            vqp = actx.enter_context(tc.tile_pool(name="vqp", bufs=1))
            bcp = actx.enter_context(tc.tile_pool(name="bcp", bufs=2))
            attn_ps = actx.enter_context(tc.tile_pool(name="attn_ps", bufs=2, space="PSUM"))
            u_ps = actx.enter_context(tc.tile_pool(name="u_ps", bufs=2, space="PSUM"))
            expp = actx.enter_context(tc.tile_pool(name="expp", bufs=1))
            maskp = actx.enter_context(tc.tile_pool(name="maskp", bufs=1))
            outp = actx.enter_context(tc.tile_pool(name="outp", bufs=2))

            def bcast_row(row_f32r, tag):
                """Broadcast a [1, HD] f32r row across 128 partitions via K=1 matmul."""
                ps = ppool.tile([P, HD], F32, tag="proj")
                nc.tensor.matmul(ps, ones_all[0:1, :], row_f32r, start=True, stop=True)
                bc = bcp.tile([P, HD], F32, tag=tag)
                nc.scalar.copy(bc, ps)
                return bc

            for b in range(B):
                qT = qkT.tile([P, 2, T], F32R, tag="qT")
                kT = qkT.tile([P, 2, T], F32R, tag="kT")
                kraw = bigp.tile([P, CPB, HD], F32, tag="kraw")
                vraw = bigp.tile([P, CPB, HD], F32, tag="vraw")
                vq = vqp.tile([P, CPB, HL, D + 2], F32R, tag="vq")
                nc.scalar.copy(vq[:, :, :, D:D + 1],
                               ones_f32[:, None, None, :].to_broadcast((P, CPB, HL, 1)))
                # per-channel stat accumulators (hd on partitions, m subtile on free)
                scol = spb.tile([P, 2], F32, tag="scol")
                vmaxc = spb.tile([P, 2], F32, tag="vmaxc")
                vminc = spb.tile([P, 2], F32, tag="vminc")

                def transpose_to(src, dst, ml):
                    # src [128, HD] tokens x headdim -> dst [128, 2, T] slices
                    for sub in range(2):
                        pt = tpsum.tile([P, P], F32, tag="pt")
                        nc.tensor.transpose(pt, src[:, P * sub: P * sub + P], ident)
                        nc.scalar.copy(dst[:, sub, P * ml: P * ml + P], pt)

                for ml in range(CPB):
                    m = b * CPB + ml
                    hst = hsp.tile([P, KSUB, P], F32R, tag="hst")
                    nc.sync.dma_start(hst, hs_re[m])
                    psums = []
                    for i in range(3):
                        ps = ppool.tile([P, HD], F32, tag="proj")
                        for ko in range(KSUB):
                            nc.tensor.matmul(ps, hst[:, ko, :], w_sb[i][:, ko, :],
                                             start=(ko == 0), stop=False)
                        nc.tensor.matmul(ps, ones_all[0:1, :], b_sb[i],
                                         start=False, stop=True)
                        psums.append(ps)
                    # Q: quantize per token now, transpose into qT
                    qraw = tmp.tile([P, HD], F32, tag="qraw")
                    nc.scalar.copy(qraw, psums[0])
                    _per_token_quant(nc, spc, qraw)
                    transpose_to(qraw, qT, ml)
                    # K, V: stash raw
                    nc.scalar.copy(kraw[:, ml, :], psums[1])
                    nc.scalar.copy(vraw[:, ml, :], psums[2])
                    # channel stats via PE transpose + psum reduce
                    for src, stats in ((kraw, "k"), (vraw, "v")):
                        pts = ppool.tile([P, HD], F32, tag="proj")
                        for sub in range(2):
                            nc.tensor.transpose(pts[:, P * sub: P * sub + P],
                                                src[:, ml, P * sub: P * sub + P], ident)
                        ptv = pts.rearrange("p (s t) -> p s t", t=P)
                        if stats == "k":
                            pk = spc.tile([P, 2], F32, tag="pk")
                            nc.vector.tensor_reduce(pk, ptv, AX.X, OP.max,
                                                    apply_absolute_value=True)
                            if ml == 0:
                                nc.vector.tensor_copy(scol, pk)
                            else:
                                nc.vector.tensor_tensor(scol, scol, pk, OP.max)
                        else:
                            pvx = spc.tile([P, 2], F32, tag="pvx")
                            pvn = spc.tile([P, 2], F32, tag="pvn")
                            nc.vector.tensor_reduce(pvx, ptv, AX.X, OP.max)
                            nc.vector.tensor_reduce(pvn, ptv, AX.X, OP.min)
                            if ml == 0:
                                nc.vector.tensor_copy(vmaxc, pvx)
                                nc.vector.tensor_copy(vminc, pvn)
                            else:
                                nc.vector.tensor_tensor(vmaxc, vmaxc, pvx, OP.max)
                                nc.vector.tensor_tensor(vminc, vminc, pvn, OP.min)

                # flatten stat columns [128, 2] -> rows via sbuf->dram->sbuf DMA
                rows = []
                for i, col in enumerate((scol, vmaxc, vminc)):
                    nc.sync.dma_start(scratch[i].rearrange("(m p) -> p m", p=P), col)
                    rw = spb.tile([1, HD], F32, tag=f"row{i}")
                    nc.sync.dma_start(rw, scratch[i: i + 1])
                    rows.append(rw)
                srow = spb.tile([1, HD], F32, tag="srow")
                nc.vector.tensor_scalar(srow, rows[0], 1e-5, None, OP.max)
                rs_row = spb.tile([1, HD], F32R, tag="rs_row")
                with nc.allow_low_precision(reason="f32r is 4-byte; rounding benign"):
                    nc.vector.reciprocal(rs_row, srow)
                srow_r = spb.tile([1, HD], F32R, tag="srow_r")
                nc.vector.tensor_copy(srow_r, srow)
                bc_rs = bcast_row(rs_row, "bc_rs")
                bc_s = bcast_row(srow_r, "bc_s")

                vmaxr = spb.tile([1, HD], F32, tag="vmaxr")
                vminr = spb.tile([1, HD], F32, tag="vminr")
                nc.vector.tensor_copy(vmaxr, rows[1])
                nc.vector.tensor_copy(vminr, rows[2])
                vrs, vscale, vnegz, vmhz = _quant_params(nc, spb, vmaxr, vminr,
                                                         [1, HD], "vq")
                bc_v = []
                for nm, t in (("bc_vrs", vrs), ("bc_vscale", vscale),
                              ("bc_vnegz", vnegz), ("bc_vmhz", vmhz)):
                    tr = spb.tile([1, HD], F32R, tag=f"{nm}_r")
                    nc.vector.tensor_copy(tr, t)
                    bc_v.append(bcast_row(tr, nm))
                bc_vrs, bc_vscale, bc_vnegz, bc_vmhz = bc_v

                for ml in range(CPB):
                    # K pass 2: smooth, quantize, un-smooth, transpose into kT
                    kt = tmp.tile([P, HD], F32, tag="ktmp")
                    nc.vector.tensor_tensor(kt, kraw[:, ml, :], bc_rs, OP.mult)
                    _per_token_quant(nc, spc, kt)
                    nc.vector.tensor_tensor(kt, kt, bc_s, OP.mult)
                    transpose_to(kt, kT, ml)
                    # V pass 2: per-channel quantize into vq (cols 0..63 per head)
                    vdst = vq[:, ml, :, 0:D]
                    vsrc = vraw[:, ml, :].rearrange("p (h d) -> p h d", d=D)
                    bview = lambda t: t.rearrange("p (h d) -> p h d", d=D)
                    vtmp = tmp.tile([P, HL, D], F32, tag="vtmp")
                    nc.vector.tensor_tensor(vtmp, vsrc, bview(bc_vrs), OP.mult)
                    nc.vector.tensor_scalar(vtmp, vtmp, MAGIC, MAGIC, OP.add, OP.subtract)
                    nc.vector.tensor_tensor(vtmp, vtmp, bview(bc_vnegz), OP.max)
                    nc.vector.tensor_tensor(vtmp, vtmp, bview(bc_vmhz), OP.min)
                    nc.vector.tensor_tensor(vdst, vtmp, bview(bc_vscale), OP.mult)

                # attention for this batch
                mask_sb = maskp.tile([P, CPB, P], F32, tag="mask_sb")
                nc.sync.dma_start(mask_sb, maskband[b].rearrange("s p n -> p s n"))

                for h in range(HL):
                    po = D * (h % 2)
                    mi = h // 2
                    kTh = kT[po: po + D, mi, :]
                    qTh = qT[po: po + D, mi, :]
                    expT = expp.tile([P, 12, 512], F32R, tag="expT")
                    for J in range(2):
                        qs = slice(512 * J, 512 * J + 512)
                        smax = 4 * J + 3
                        eoff = 4 * J  # J=0 -> blocks 0..3, J=1 -> blocks 4..11
                        for s in range(smax + 1):
                            psc = attn_ps.tile([P, 512], F32, tag="psc")
                            nc.tensor.matmul(psc, kTh[:, P * s: P * s + P],
                                             qTh[:, qs], start=True, stop=True)
                            if s >= 4 * J:
                                c0 = P * (s - 4 * J)
                                nc.vector.tensor_tensor(psc[:, c0:c0 + P],
                                                        psc[:, c0:c0 + P],
                                                        mask_sb[:, s, :], OP.add)
                                if c0 > 0:
                                    nc.vector.tensor_copy(expT[:, eoff + s, 0:c0],
                                                          zeros_f32[:, 0:c0])
                                nc.scalar.activation(expT[:, eoff + s, c0:],
                                                     psc[:, c0:], ACTF.Exp)
                            else:
                                nc.scalar.activation(expT[:, eoff + s, :], psc, ACTF.Exp)
                        U = u_ps.tile([P, 512], F32, tag="U")
                        for s in range(smax + 1):
                            nc.tensor.matmul(U[0: D + 1, :], vq[:, s, h, 0: D + 1],
                                             expT[:, eoff + s, :],
                                             start=(s == 0), stop=(s == smax))
                        # denominator: recip at row D, broadcast via K=1 matmul
                        rrow = dpool.tile([P, 512], F32R, tag="rrow")
                        with nc.allow_low_precision(reason="f32r recip benign"):
                            nc.vector.reciprocal(rrow[D: D + 1, :], U[D: D + 1, :])
                        psb = attn_ps.tile([P, 512], F32, tag="psc")
                        nc.tensor.matmul(psb[0:D, :], ones_all[D: D + 1, 0:D],
                                         rrow[D: D + 1, :], start=True, stop=True)
                        dbc = dpool.tile([D, 512], F32, tag="dbc")
                        nc.scalar.copy(dbc, psb[0:D, :])
                        attn_sb = outp.tile([D, 512], F32R, tag="attn_sb")
                        nc.vector.tensor_tensor(attn_sb, U[0:D, :], dbc, OP.mult)
                        nc.sync.dma_start(a2a_in[2 * b + J, D * h: D * h + D, :], attn_sb)

        nc.gpsimd.collective_compute(
            "AllToAll",
            OP.bypass,
            ins=[a2a_in[:].opt()],
            outs=[a2a_out[:].opt()],
            replica_groups=[list(range(NCORES))],
        )

        # out projection on this core's 512-token slice
        with ExitStack() as octx:
            rhp = octx.enter_context(tc.tile_pool(name="rhp", bufs=1))
            wop = octx.enter_context(tc.tile_pool(name="wop", bufs=3))
            ops = octx.enter_context(tc.tile_pool(name="ops", bufs=2, space="PSUM"))
            osb = octx.enter_context(tc.tile_pool(name="osb", bufs=2))

            rhs_sb = rhp.tile([P, KSUB, TSLICE], F32R, tag="rhs_sb")
            nc.sync.dma_start(
                rhs_sb, a2a_out[:].rearrange("c (k2 p) n -> p (c k2) n", p=P)
            )
            for mo in range(KSUB):
                wot = wop.tile([P, KSUB, P], F32R, tag="wot")
                nc.sync.dma_start(wot, wo_re[mo])
                ps = ops.tile([P, TSLICE], F32, tag="ops")
                for ko in range(KSUB):
                    nc.tensor.matmul(ps, wot[:, ko, :], rhs_sb[:, ko, :],
                                     start=(ko == 0), stop=(ko == KSUB - 1))
                ob = osb.tile([P, TSLICE], F32, tag="ob")
                nc.scalar.activation(ob, ps, ACTF.Identity, bias=bo_sb[:, mo: mo + 1])
                nc.sync.dma_start(out_dram[mo], ob)


_NC_CACHE = None


def _get_nc():
    global _NC_CACHE
    if _NC_CACHE is None:
        _NC_CACHE = build_nc()
    return _NC_CACHE


def make_in_maps(inputs):
    hs = np.ascontiguousarray(np.asarray(inputs["hidden_states"], np.float32)).reshape(TOK, E)
    wq = np.asarray(inputs["wq"], np.float32)
    wk = np.asarray(inputs["wk"], np.float32)
    wv = np.asarray(inputs["wv"], np.float32)
    wo = np.asarray(inputs["wo"], np.float32)
    bq = np.asarray(inputs["bq"], np.float32)
    bk = np.asarray(inputs["bk"], np.float32)
    bv = np.asarray(inputs["bv"], np.float32)
    bo = np.asarray(inputs["bo"], np.float32)
    am = np.asarray(inputs["attention_mask"], np.float32)

    scaling = np.float32(D ** -0.5)

    # hs_re[m, p, ko, t] = hsT[ko*128+p, m*128+t]
    hsT = np.ascontiguousarray(hs.T)  # [E, TOK]
    hs_re = np.ascontiguousarray(
        hsT.reshape(KSUB, P, NCHUNK, P).transpose(2, 1, 0, 3)
    )
    # wo_re[mo, p, ko, mhat] = woT[ko*128+p, mo*128+mhat]
    woT = np.ascontiguousarray(wo.T)
    wo_re = np.ascontiguousarray(
        woT.reshape(KSUB, P, KSUB, P).transpose(2, 1, 0, 3)
    )
    bo_re = np.ascontiguousarray(bo.reshape(KSUB, P).T)

    # mask diag blocks (broadcast batch dim if needed)
    maskband = np.empty((B, CPB, P, P), np.float32)
    for b in range(B):
        mb = am[min(b, am.shape[0] - 1), 0].T  # [k, q]
        for s in range(CPB):
            maskband[b, s] = mb[P * s: P * s + P, P * s: P * s + P]

    in_maps = []
    for c in range(NCORES):
        rows = slice(HD * c, HD * c + HD)
        in_maps.append(dict(
            hs_re=hs_re,
            wq=np.ascontiguousarray((wq[rows] * scaling).T.reshape(KSUB, P, HD)),
            wk=np.ascontiguousarray(wk[rows].T.reshape(KSUB, P, HD)),
            wv=np.ascontiguousarray(wv[rows].T.reshape(KSUB, P, HD)),
            bqr=np.ascontiguousarray((bq[rows] * scaling).reshape(1, HD)),
            bkr=np.ascontiguousarray(bk[rows].reshape(1, HD)),
            bvr=np.ascontiguousarray(bv[rows].reshape(1, HD)),
            wo_re=wo_re,
            bo_re=bo_re,
            maskband=maskband,
        ))
    return in_maps


def assemble_output(per_core_outs):
    """per_core_outs: list of 8 arrays [KSUB, P, TSLICE] (out.T slices)."""
    outT = np.concatenate(
        [np.asarray(o).reshape(E, TSLICE) for o in per_core_outs], axis=1
    )  # [E, TOK]
    return np.ascontiguousarray(outT.T).reshape(B, T, E)


def kernel(**inputs):
    from concourse import bass_utils
    nc = _get_nc()
    in_maps = make_in_maps(inputs)
    res = bass_utils.run_bass_kernel_spmd(nc, in_maps, core_ids=list(range(NCORES)))
    return assemble_output([res.results[c]["out"] for c in range(NCORES)])
